# revision 36
# baseline (speedup 1.0000x reference)
"""Trainium2 Bass kernel for windowed attention with LoRA + decomposed rel-pos bias.

Full-input contract: kernel(**inputs) takes the unsharded numpy inputs and
returns the full (64, 14, 14, 768) float32 output.

Strategy (8 NeuronCores, data-parallel over the 64-window batch, 8 windows/core):
  Host prep (numpy):
    - Fold LoRA into qkv weights:  Wq += lb_q@la_q, Wv += lb_v@la_v  (exact math).
    - Fold attention scale (2^-3, exact) into Wq / b_q; rel-pos tables get 1/scale.
    - Pre-transpose all weights + x so every on-chip matmul operand has its
      contraction dim on SBUF partitions (no on-chip transposes at all).
    - Rel-pos tables zero-padded to 32-wide g-blocks so the rel matmuls also
      write the zero gap rows of the augmented q tiles (no zero-fill DMAs).
    - One-hot key tables pre-replicated across the 48 pair slots so the fill
      DMAs are fully contiguous per partition (no 2x small-line penalty).
  On chip (per core, all SBUF resident):
    - qk projection -> per-(window, head-pair) parity-split "augmented" q/k
      tiles [128, 48, 196]: one matmul per key-chunk produces
      q@k^T*scale + rel_h + rel_w directly in PSUM (K-augmentation trick).
    - exp on ScalarE (softmax without max-subtraction: logits are O(1)).
    - attn@v with an appended ones-column on v so the softmax denominator
      falls out of the same matmul; normalize with a reciprocal + DRAM-bounce
      DMA partition-broadcast + one VectorE multiply.
    - DMA issue distributed across SP/Pool/Act queues so transfers overlap.
"""

import numpy as np
import ml_dtypes

B_TOTAL = 64
NCORES = 8
BPC = B_TOTAL // NCORES  # windows per core
H = W = 14
N = H * W  # 196 tokens per window
DIM = 768
NH = 12
HD = 64
DC = DIM // 128  # 6 contraction chunks
NHH = NH // 2  # 6 head pairs
NKT0, NKT1 = 128, N - 128  # key-token chunks (128 + 68)
SCALE = HD ** -0.5  # 0.125, exact power of two

# augmented-tile row maps (parity-split):
#  even heads (qaug_e/kaug_e): q/k rows 0:64, relh feats+zeros 64:96,
#    relw feats 96:110 (+junk-zeros 110:128); contraction [0:110)
#  odd heads (qaug_o/kaug_o): relw feats+zeros 0:32, relh feats+zeros 32:64,
#    q/k rows 64:128; contraction [0:128)
K_EVEN = 110
K_ODD = 128
GPAD = 32  # zero-padded rel-table g-block width

_NC_CACHE = {}


def build_module():
    from contextlib import ExitStack

    import concourse.tile as tile
    from concourse import bacc, mybir

    f32 = mybir.dt.float32
    bf16 = mybir.dt.bfloat16
    AF = mybir.ActivationFunctionType
    ALU = mybir.AluOpType

    nc = bacc.Bacc(
        "TRN2", target_bir_lowering=False, debug=False, num_devices=NCORES
    )

    T = BPC * N  # 1568 tokens per core

    xT = nc.dram_tensor("xT", [DIM, T], bf16, kind="ExternalInput").ap()
    wqk = nc.dram_tensor("wqk", [12 * 128, DIM], bf16, kind="ExternalInput").ap()
    wv = nc.dram_tensor("wv", [DIM, DIM], bf16, kind="ExternalInput").ap()
    pw = nc.dram_tensor("pw", [DIM, DIM], bf16, kind="ExternalInput").ap()
    bqk = nc.dram_tensor("bqk", [2 * DIM], f32, kind="ExternalInput").ap()
    bv = nc.dram_tensor("bv", [DIM], bf16, kind="ExternalInput").ap()
    bp = nc.dram_tensor("bp", [DIM], bf16, kind="ExternalInput").ap()
    relh = nc.dram_tensor("relh", [HD, H * GPAD], bf16, kind="ExternalInput").ap()
    relw = nc.dram_tensor("relw", [HD, W * GPAD], bf16, kind="ExternalInput").ap()
    oh_e = nc.dram_tensor("oh_e", [46, 48 * N], bf16, kind="ExternalInput").ap()
    oh_o = nc.dram_tensor("oh_o", [64, 48 * N], bf16, kind="ExternalInput").ap()
    out = nc.dram_tensor("out", [T, DIM], f32, kind="ExternalOutput").ap()

    with tile.TileContext(nc) as tc, ExitStack() as ctx:
        singles = ctx.enter_context(tc.tile_pool(name="singles", bufs=1))
        ps = ctx.enter_context(tc.tile_pool(name="ps", bufs=2, space="PSUM"))
        pa_pool = ctx.enter_context(tc.tile_pool(name="pa", bufs=2, space="PSUM"))
        psd_cm = tc.tile_pool(name="psd", bufs=2, space="PSUM")
        psd = psd_cm.__enter__()
        attn_pool = ctx.enter_context(tc.tile_pool(name="attn", bufs=3))
        osb_pool = ctx.enter_context(tc.tile_pool(name="osb", bufs=2))
        r_pool = ctx.enter_context(tc.tile_pool(name="rp", bufs=4))
        xt_pool_cm = tc.tile_pool(name="xt", bufs=1)
        xt_pool = xt_pool_cm.__enter__()

        # ---- resident SBUF tensors; DMA order = first-use order, spread
        #      across SP / Pool / Act issue queues ----
        wqk_sb = singles.tile([128, 12, DC, 128], bf16)
        wqk_r = wqk.rearrange("(oc p) k -> p oc k", p=128)
        bqk_sb = singles.tile([128, 2 * DC], f32)
        xT_sb = xt_pool.tile([128, DC, T], bf16)
        xT_r = xT.rearrange("(c p) t -> p c t", p=128)

        # one extra pad slot on the k side: chunk-1 QK matmuls read 128 key
        # columns (68 real + 60 finite pad) so their psum rows are all
        # written and a single exp can cover the whole tile
        qaug_e = singles.tile([128, BPC * NHH, N], bf16)
        qaug_o = singles.tile([128, BPC * NHH, N], bf16)
        kaug_e = singles.tile([128, BPC * NHH + 1, N], bf16)
        kaug_o = singles.tile([128, BPC * NHH + 1, N], bf16)
        nc.vector.memset(kaug_e[:, BPC * NHH, :], 0.0)
        nc.vector.memset(kaug_o[:, BPC * NHH, :], 0.0)

        # first matmul group needs wqk oc=0 + xT b2=0 only
        nc.sync.dma_start(
            out=wqk_sb[:, 0, :, :].rearrange("p c j -> p (c j)"),
            in_=wqk_r[:, 0, :],
        )
        nc.sync.dma_start(out=bqk_sb[:], in_=bqk.rearrange("(c p) -> p c", p=128))
        for c in range(DC):
            nc.sync.dma_start(
                out=xT_sb[:, c, 0:392], in_=xT_r[:, c, 0:392]
            )
        for oc in range(1, 12):
            nc.sync.dma_start(
                out=wqk_sb[:, oc, :, :].rearrange("p c j -> p (c j)"),
                in_=wqk_r[:, oc, :],
            )
        for b2 in range(1, 4):
            for c in range(DC):
                nc.sync.dma_start(
                    out=xT_sb[:, c, b2 * 392 : (b2 + 1) * 392],
                    in_=xT_r[:, c, b2 * 392 : (b2 + 1) * 392],
                )

        # one-hot key tables (pre-replicated, contiguous): Pool + Act queues
        nc.gpsimd.dma_start(
            out=kaug_e[64:110, 0:24, :].rearrange("p s q -> p (s q)"),
            in_=oh_e[:, 0 : 24 * N],
        )
        nc.scalar.dma_start(
            out=kaug_e[64:110, 24:48, :].rearrange("p s q -> p (s q)"),
            in_=oh_e[:, 24 * N :],
        )
        nc.gpsimd.dma_start(
            out=kaug_o[0:64, 0:24, :].rearrange("p s q -> p (s q)"),
            in_=oh_o[:, 0 : 24 * N],
        )
        nc.scalar.dma_start(
            out=kaug_o[0:64, 24:48, :].rearrange("p s q -> p (s q)"),
            in_=oh_o[:, 24 * N :],
        )

        relh_sb = singles.tile([128, H * GPAD], bf16)
        nc.gpsimd.dma_start(out=relh_sb[0:64, :], in_=relh)
        nc.gpsimd.dma_start(out=relh_sb[64:128, :], in_=relh)
        relw_sb = singles.tile([128, W * GPAD], bf16)
        nc.gpsimd.dma_start(out=relw_sb[0:64, :], in_=relw)
        nc.gpsimd.dma_start(out=relw_sb[64:128, :], in_=relw)

        bv_sb = singles.tile([128, DIM], bf16)
        nc.sync.dma_start(out=bv_sb[:], in_=bv.unsqueeze(0).broadcast_to([128, DIM]))
        wv_sb = singles.tile([128, DC, DIM], bf16)
        nc.sync.dma_start(out=wv_sb[:], in_=wv.rearrange("(c p) o -> p c o", p=128))
        pw_sb = singles.tile([128, DC, DIM], bf16)
        nc.gpsimd.dma_start(out=pw_sb[:], in_=pw.rearrange("(c p) o -> p c o", p=128))
        bp_sb = singles.tile([128, DIM], bf16)
        nc.gpsimd.dma_start(out=bp_sb[:], in_=bp.unsqueeze(0).broadcast_to([128, DIM]))

        # [t-chunk partitions, window, chunk, head, hd+ones]
        vall = singles.tile([128, BPC, 2, NH, HD + 1], bf16)
        nc.vector.memset(vall[:, :, :, :, HD : HD + 1], 1.0)

        # views (slot = b*NHH + hh, b-major)
        qv_e = qaug_e.rearrange("p (b hh) q -> p b hh q", b=BPC)
        qv_o = qaug_o.rearrange("p (b hh) q -> p b hh q", b=BPC)
        qv6_e = qaug_e.rearrange(
            "p (b hh) (qh qw) -> p b hh qh qw", b=BPC, qh=H
        )
        qv6_o = qaug_o.rearrange(
            "p (b hh) (qh qw) -> p b hh qh qw", b=BPC, qh=H
        )

        # ---- phase 1: q/k projection (two windows per psum tile) ----
        dest_q = [
            qaug_e.rearrange("p (b2 w2 hh) q -> p b2 w2 hh q", w2=2, hh=NHH),
            qaug_o.rearrange("p (b2 w2 hh) q -> p b2 w2 hh q", w2=2, hh=NHH),
        ]
        dest_k = [
            kaug_e[:, 0 : BPC * NHH, :].rearrange(
                "p (b2 w2 hh) q -> p b2 w2 hh q", w2=2, hh=NHH
            ),
            kaug_o[:, 0 : BPC * NHH, :].rearrange(
                "p (b2 w2 hh) q -> p b2 w2 hh q", w2=2, hh=NHH
            ),
        ]
        for b2 in range(BPC // 2):
            for oc in range(2 * DC):  # 6 q chunks then 6 k chunks
                p_qk = ps.tile([128, 512], f32, tag="ps")
                for dc in range(DC):
                    nc.tensor.matmul(
                        p_qk[:, 0 : 2 * N],
                        lhsT=wqk_sb[:, oc, dc, :],
                        rhs=xT_sb[:, dc, 2 * b2 * N : (2 * b2 + 2) * N],
                        start=(dc == 0),
                        stop=(dc == DC - 1),
                    )
                is_q = oc < DC
                hh = oc % DC
                dvs = dest_q if is_q else dest_k
                # evictions split across Act (par0) and DVE (par1)
                nc.scalar.activation(
                    out=dvs[0][0:64, b2, :, hh, :],
                    in_=p_qk[0:64, 0 : 2 * N].rearrange("p (w q) -> p w q", w=2),
                    func=AF.Identity,
                    bias=bqk_sb[0:64, oc : oc + 1],
                    scale=1.0,
                )
                nc.vector.tensor_scalar_add(
                    out=dvs[1][64:128, b2, :, hh, :],
                    in0=p_qk[64:128, 0 : 2 * N].rearrange("p (w q) -> p w q", w=2),
                    scalar1=bqk_sb[64:128, oc : oc + 1],
                )

        # ---- rel features (all heads) interleaved with the v projection ----
        # rel-feat copybacks rotate across DVE / Pool to spread load
        _copy_rr = [0]

        def rel_copy(out_ap, in_ap):
            # GPSIMD cannot touch PSUM on HW: rotate DVE / Act only
            k = _copy_rr[0] % 2
            _copy_rr[0] += 1
            if k == 0:
                nc.vector.tensor_copy(out=out_ap, in_=in_ap)
            else:
                nc.scalar.activation(
                    out=out_ap, in_=in_ap, func=AF.Copy, scale=1.0
                )

        def emit_rel(hx):
            par = hx % 2
            hh = hx // 2
            if par == 0:
                q_rows, relh_tp, relw_tp = slice(0, 64), (0, 64), (0, 96)
                relh_rows, relw_rows = slice(64, 96), slice(96, 128)
                qv, qv6 = qv_e, qv6_e
            else:
                q_rows, relh_tp, relw_tp = slice(64, 128), (64, 32), (64, 0)
                relh_rows, relw_rows = slice(32, 64), slice(0, 32)
                qv, qv6 = qv_o, qv6_o
            for g0 in range(0, H, 4):  # quarters of 4 (last has 2)
                gn = min(4, H - g0)
                p_r = psd.tile([128, 4, 128], f32, tag="psd")
                for s in range(gn):
                    g = g0 + s
                    nc.tensor.matmul(
                        p_r[relh_rows, s, 0 : BPC * W],
                        lhsT=relh_sb[q_rows, g * GPAD : (g + 1) * GPAD],
                        rhs=qv[q_rows, :, hh, g * W : (g + 1) * W],
                        start=True,
                        stop=True,
                        tile_position=relh_tp,
                    )
                    nc.tensor.matmul(
                        p_r[relw_rows, s, 0 : BPC * W],
                        lhsT=relw_sb[q_rows, g * GPAD : (g + 1) * GPAD],
                        rhs=qv[q_rows, :, hh, g : g + 13 * W + 1 : W],
                        start=True,
                        stop=True,
                        tile_position=relw_tp,
                    )
                rel_copy(
                    qv6[relh_rows, :, hh, g0 : g0 + gn, :],
                    p_r[relh_rows, 0:gn, 0 : BPC * W].rearrange(
                        "p s (b w) -> p b s w", b=BPC
                    ),
                )
                rel_copy(
                    qv6[relw_rows, :, hh, :, g0 : g0 + gn],
                    p_r[relw_rows, 0:gn, 0 : BPC * W].rearrange(
                        "p s (b q) -> p b q s", b=BPC
                    ),
                )

        def emit_v(b, i, half):
            tc_rows = NKT0 if i == 0 else NKT1
            t0 = b * N + i * 128
            p_v = ps.tile([128, 512], f32, tag="ps")
            for dc in range(DC):
                nc.tensor.matmul(
                    p_v[0:tc_rows, 0:384],
                    lhsT=xT_sb[:, dc, t0 : t0 + tc_rows],
                    rhs=wv_sb[:, dc, half * 384 : (half + 1) * 384],
                    start=(dc == 0),
                    stop=(dc == DC - 1),
                )
            nc.vector.tensor_tensor(
                out=vall[0:tc_rows, b, i, 6 * half : 6 * half + 6, 0:HD],
                in0=p_v[0:tc_rows, 0:384].rearrange("p (h d) -> p h d", h=6),
                in1=bv_sb[0:tc_rows, half * 384 : (half + 1) * 384].rearrange(
                    "p (h d) -> p h d", h=6
                ),
                op=ALU.add,
            )

        def emit_qk(b, hx, p_a, sl):
            # chunk-1 lhsT extended to 128 key columns (spills into the next
            # slot / pad slot) so all 128 psum rows are written -> one exp
            par = hx % 2
            hh = hx // 2
            slot = b * NHH + hh
            if par == 0:
                qp_t, kp_t, krange = qaug_e, kaug_e, slice(0, K_EVEN)
            else:
                qp_t, kp_t, krange = qaug_o, kaug_o, slice(0, K_ODD)
            kp_flat = kp_t.rearrange("p s q -> p (s q)")
            nc.tensor.matmul(
                p_a[:, sl, 0:N],
                lhsT=kp_t[krange, slot, 0:NKT0],
                rhs=qp_t[krange, slot, :],
                start=True,
                stop=True,
            )
            nc.tensor.matmul(
                p_a[:, sl + 1, 0:N],
                lhsT=kp_flat[krange, slot * N + NKT0 : slot * N + NKT0 + 128],
                rhs=qp_t[krange, slot, :],
                start=True,
                stop=True,
            )

        def emit_av(b, hx, a_sb, sl, p_o):
            # AV with the ones column folded in: psum rows 0:64 = hd values,
            # row 64 = softmax denominator. par1 uses the spare psum columns.
            par = hx % 2
            c0 = 0 if par == 0 else 256
            nc.tensor.matmul(
                p_o[0:65, c0 : c0 + N],
                lhsT=vall[0:NKT0, b, 0, hx, 0 : HD + 1],
                rhs=a_sb[:, sl, :],
                start=True,
                stop=False,
                skip_group_check=True,
            )
            nc.tensor.matmul(
                p_o[0:65, c0 : c0 + N],
                lhsT=vall[0:NKT1, b, 1, hx, 0 : HD + 1],
                rhs=a_sb[0:NKT1, sl + 1, :],
                start=False,
                stop=True,
                skip_group_check=True,
            )

        # ---- rel features + v projection, interleaved ----
        v_groups = [
            (b, i, half) for b in range(BPC) for i in range(2) for half in range(2)
        ]
        vg_it = iter(v_groups)
        for hx in range(NH):
            for _ in range(3):
                g = next(vg_it, None)
                if g is not None:
                    emit_v(*g)
            emit_rel(hx)
        for g in vg_it:
            emit_v(*g)

        # xT no longer needed; free its zone for o2_all. rel psum pool freed
        # for the projection psum pool.
        xt_pool_cm.__exit__(None, None, None)
        psd_cm.__exit__(None, None, None)
        o2_pool = ctx.enter_context(tc.tile_pool(name="o2", bufs=1))
        o2_all = o2_pool.tile([128, DC, T], bf16)
        pp_pool = ctx.enter_context(tc.tile_pool(name="pp", bufs=2, space="PSUM"))

        # ---- attention, window-outer + software-pipelined, with the
        #      output projection interleaved per 128-token chunk ----
        def emit_qk_exp(b, hh):
            p_a = pa_pool.tile([128, 4, 256], f32, tag="pa")
            a_sb = attn_pool.tile([128, 4, N], bf16, tag="a")
            emit_qk(b, 2 * hh, p_a, 0)
            emit_qk(b, 2 * hh + 1, p_a, 2)
            nc.scalar.activation(
                out=a_sb[:, :, :], in_=p_a[:, :, 0:N], func=AF.Exp, scale=1.0
            )
            return a_sb

        def emit_av_norm(b, hh, a_sb):
            p_o = ps.tile([128, 512], f32, tag="ps")
            emit_av(b, 2 * hh, a_sb, 0, p_o)
            emit_av(b, 2 * hh + 1, a_sb, 2, p_o)
            rr = r_pool.tile([65, 2, N], bf16, tag="rr")
            rin = p_o[64:65, :].rearrange("p (s c) -> p s c", s=2)[:, :, 0:N]
            with nc.allow_low_precision(reason="bf16 softmax recip"):
                nc.vector.reciprocal(out=rr[64:65, :, :], in_=rin)
            rb = r_pool.tile([64, 2, N], bf16, tag="rb")
            nc.gpsimd.partition_broadcast(rb[:, 0, :], rr[64:65, 0, :], channels=64)
            nc.gpsimd.partition_broadcast(rb[:, 1, :], rr[64:65, 1, :], channels=64)
            nc.vector.tensor_tensor(
                out=o2_all[0:64, hh, b * N : (b + 1) * N],
                in0=p_o[0:64, 0:N],
                in1=rb[:, 0, :],
                op=ALU.mult,
            )
            nc.vector.tensor_tensor(
                out=o2_all[64:128, hh, b * N : (b + 1) * N],
                in0=p_o[0:64, 256 : 256 + N],
                in1=rb[:, 1, :],
                op=ALU.mult,
            )

        def emit_proj_chunk(j):
            t0 = j * 128
            tc_rows = min(128, T - t0)
            o_sb = osb_pool.tile([128, DIM], f32, tag="osb")
            for half in range(2):
                p_p = pp_pool.tile([128, 512], f32, tag="pp")
                for cc in range(DC):
                    nc.tensor.matmul(
                        p_p[0:tc_rows, 0:384],
                        lhsT=o2_all[:, cc, t0 : t0 + tc_rows],
                        rhs=pw_sb[:, cc, half * 384 : (half + 1) * 384],
                        start=(cc == 0),
                        stop=(cc == DC - 1),
                    )
                nc.vector.tensor_tensor(
                    out=o_sb[0:tc_rows, half * 384 : (half + 1) * 384],
                    in0=p_p[0:tc_rows, 0:384],
                    in1=bp_sb[0:tc_rows, half * 384 : (half + 1) * 384],
                    op=ALU.add,
                )
            nc.sync.dma_start(
                out=out[t0 : t0 + tc_rows, :],
                in_=o_sb[0:tc_rows, :],
            )

        NT_CH = (T + 127) // 128  # 13
        # proj chunk j is ready once window (j+1)*128-1 // 196 is normalized;
        # delay emission by a few pairs so the evictions have drained
        DELAY = 3
        pairs = [(b, hh) for b in range(BPC) for hh in range(NHH)]
        chunk_ready = {}  # emission index -> list of chunks
        next_chunk = 0
        for b in range(BPC):
            done_tokens = (b + 1) * N
            last_pair_idx = b * NHH + (NHH - 1)
            ready = []
            while next_chunk < NT_CH and (next_chunk + 1) * 128 <= done_tokens:
                ready.append(next_chunk)
                next_chunk += 1
            if b == BPC - 1:
                ready.append(NT_CH - 1)
            if ready:
                chunk_ready.setdefault(last_pair_idx + DELAY, []).extend(ready)

        prev = None
        for i, (b, hh) in enumerate(pairs):
            a_sb = emit_qk_exp(b, hh)
            if prev is not None:
                emit_av_norm(prev[0], prev[1], prev[2])
            prev = (b, hh, a_sb)
            for j in chunk_ready.get(i, ()):
                emit_proj_chunk(j)
        emit_av_norm(prev[0], prev[1], prev[2])
        for i in sorted(k for k in chunk_ready if k >= len(pairs)):
            for j in chunk_ready[i]:
                emit_proj_chunk(j)

    nc.finalize()
    return nc


def _host_prep(inputs):
    bf16 = ml_dtypes.bfloat16
    x = np.asarray(inputs["x"], np.float32)
    qkv_w = np.asarray(inputs["qkv_w"], np.float32)
    qkv_b = np.asarray(inputs["qkv_b"], np.float32)
    proj_w = np.asarray(inputs["proj_w"], np.float32)
    proj_b = np.asarray(inputs["proj_b"], np.float32)
    la_q = np.asarray(inputs["la_q"], np.float32)
    lb_q = np.asarray(inputs["lb_q"], np.float32)
    la_v = np.asarray(inputs["la_v"], np.float32)
    lb_v = np.asarray(inputs["lb_v"], np.float32)
    rel_pos_h = np.asarray(inputs["rel_pos_h"], np.float32)
    rel_pos_w = np.asarray(inputs["rel_pos_w"], np.float32)

    Wq = qkv_w[:DIM] + lb_q @ la_q
    Wk = qkv_w[DIM : 2 * DIM]
    Wv = qkv_w[2 * DIM :] + lb_v @ la_v

    # wqk host layout: [oc, p, c, j] with W^T[(c,p), (oc,j)] so per-oc DMA
    # lines are fully contiguous (1536B per partition)
    wqkT = np.concatenate([SCALE * Wq, Wk], 0).T  # [DIM(c,p), 2DIM(oc,j)]
    wqk_host = np.ascontiguousarray(
        wqkT.reshape(DC, 128, 12, 128)
        .transpose(2, 1, 0, 3)
        .reshape(12 * 128, DIM)
        .astype(bf16)
    )
    wv_host = np.ascontiguousarray(Wv.T.astype(bf16))
    pw_host = np.ascontiguousarray(proj_w.T.astype(bf16))
    bqk_host = np.concatenate([SCALE * qkv_b[:DIM], qkv_b[DIM : 2 * DIM]]).astype(
        np.float32
    )
    bv_host = np.ascontiguousarray(qkv_b[2 * DIM :].astype(bf16))
    bp_host = np.ascontiguousarray(proj_b.astype(bf16))

    idx = np.arange(H)[:, None] - np.arange(H)[None, :] + (H - 1)
    Rh = rel_pos_h[idx] / SCALE  # [qh, kh, hd]
    Rw = rel_pos_w[idx] / SCALE  # [qw, kw, hd]
    # zero-pad each g-block to 32 cols so the rel matmuls emit the zero rows
    relh_host = np.zeros((HD, H, GPAD), np.float32)
    relh_host[:, :, :H] = Rh.transpose(2, 0, 1)
    relw_host = np.zeros((HD, W, GPAD), np.float32)
    relw_host[:, :, :W] = Rw.transpose(2, 0, 1)
    relh_host = np.ascontiguousarray(relh_host.reshape(HD, H * GPAD).astype(bf16))
    relw_host = np.ascontiguousarray(relw_host.reshape(HD, W * GPAD).astype(bf16))

    kt = np.arange(N)
    oh_kh = (kt[None, :] // W == np.arange(H)[:, None]).astype(bf16)  # [14, 196]
    oh_kw = (kt[None, :] % W == np.arange(W)[:, None]).astype(bf16)
    z18 = np.zeros((18, N), bf16)
    oh_e_small = np.concatenate([oh_kh, z18, oh_kw], 0)  # [46, 196]
    oh_o_small = np.concatenate([oh_kw, z18, oh_kh, z18], 0)  # [64, 196]
    # replicate across the 48 pair slots -> fully contiguous fill DMAs
    oh_e_host = np.ascontiguousarray(
        np.broadcast_to(oh_e_small[:, None, :], (46, 48, N)).reshape(46, 48 * N)
    )
    oh_o_host = np.ascontiguousarray(
        np.broadcast_to(oh_o_small[:, None, :], (64, 48, N)).reshape(64, 48 * N)
    )

    shared = {
        "wqk": wqk_host,
        "wv": wv_host,
        "pw": pw_host,
        "bqk": bqk_host,
        "bv": bv_host,
        "bp": bp_host,
        "relh": relh_host,
        "relw": relw_host,
        "oh_e": oh_e_host,
        "oh_o": oh_o_host,
    }

    x_flat = x.reshape(B_TOTAL, N, DIM)
    in_maps = []
    for c in range(NCORES):
        xc = x_flat[c * BPC : (c + 1) * BPC].reshape(BPC * N, DIM)
        xT_c = np.ascontiguousarray(xc.T.astype(bf16))
        m = dict(shared)
        m["xT"] = xT_c
        in_maps.append(m)
    return in_maps


def kernel(**inputs):
    from concourse import bass_utils

    if "nc" not in _NC_CACHE:
        _NC_CACHE["nc"] = build_module()
    nc = _NC_CACHE["nc"]
    in_maps = _host_prep(inputs)
    res = bass_utils.run_bass_kernel_spmd(
        nc, in_maps, core_ids=list(range(NCORES))
    )
    outs = [r["out"].reshape(BPC, H, W, DIM) for r in res.results]
    return np.concatenate(outs, 0)
